# revision 11
# baseline (speedup 1.0000x reference)
"""Bass/Trainium2 kernel for nn_KMIPAttention (top-32 sparse attention).

B=4, S=4096, D=256, K=32. Sharding: 8 cores = (batch b = c//2) x (query half
h = c%2). Each core gets x[b] rolled so its 2048 query rows come first
(top-k/softmax/PV are permutation-invariant over the key axis), computes
out rows for those queries, host reassembles.

Per-core pipeline:
  XT = x^T via PE transposes; KT/QT = W^T-projections in [d,t] layout (fp32r
  matmuls, bias via ACT Identity+bias on the PSUM->SBUF copy); V in [t,d]
  layout with a ones column appended (free softmax denominator).
  Per q-tile [128]: sim = QK^T into PSUM, 16x vector.max over 256-chunks ->
  candidate set C[128,128] (per-chunk top-8 union), 4 rounds max/match_replace
  -> tau = 32nd largest. Per q-group [512]: simT = K@Q^T + rank-1 (-tau) via
  matmul, e = Exp(simT - tau) on ACT, pT = (e >= 0.9999)*e (DVE/GPSIMD STT),
  PV: out[q,0:256] = sum_t pT*V, out[q,256] = sum_t pT (denominator), then
  out = out[:, :256] * reciprocal(out[:,256]).
"""

import numpy as np

import concourse.bass as bass
import concourse.mybir as mybir
from concourse.tile import TileContext
from concourse.bass_utils import run_bass_kernel_spmd
from concourse.masks import make_identity
from bass_rust import ScopedClock

F32 = mybir.dt.float32
F32R = mybir.dt.float32r

S = 4096          # keys per core (full sequence of its batch)
NQ = 2048         # query rows per core
D = 256
P = 128
T_TILES = S // P          # 32
Q_TILES = NQ // P         # 16
QG = 4                    # q-tiles per group (512 q cols for simT/PV)
N_GROUPS = Q_TILES // QG  # 4
NEG_BIG = -1.0e30
MASK_THRESH = 0.9999      # e = exp(s - tau) >= ~1  <=>  s >= tau (with slack)

MAX_DRAIN_WAITS = 2


class SplitDrainTC(TileContext):
    """TileContext whose final drain splits sem waits across several drains.

    The walrus in this container rejects >MAX_DRAIN_WAITS sync waits on one
    CTRL instruction ("Too many sync wait commands"). Sync engine executes
    in order, so waits on consecutive drains are equivalent to one big one.
    """

    def _drain_and_barrier(self, tick_clock, wait_clock):
        nc = self.nc
        drain_inst = nc.sync.drain()
        wait_clock.add_sem_waits(
            drain_inst.ins, ScopedClock({None: tick_clock.global_clock})
        )
        under = drain_inst.ins
        si = under.sync_info
        waits = list(si.on_wait or []) if si is not None else []
        if len(waits) > MAX_DRAIN_WAITS:
            si.on_wait = waits[:MAX_DRAIN_WAITS]
            for i in range(MAX_DRAIN_WAITS, len(waits), MAX_DRAIN_WAITS):
                extra = nc.sync.drain()
                eu = extra.ins
                esi = eu.sync_info
                if esi is None:
                    eu.sync_info = mybir.SyncInfo(
                        on_wait=waits[i : i + MAX_DRAIN_WAITS], on_update=[]
                    )
                else:
                    esi.on_wait = waits[i : i + MAX_DRAIN_WAITS]
        nc.all_engine_barrier()
        popped = nc._tile_sem_poison_stack.pop()
        assert popped is self._sem_poison
        nc.clear_and_free_semaphores(list(self.sems.allocated().values()))
        nc.all_engine_barrier()


def _r(ap):
    """fp32r (FP22-truncated full-rate matmul) view of an fp32 AP."""
    return ap if ap.dtype == F32R else ap.bitcast(F32R)


def _split_excess_waits(nc, max_waits=1):
    """Walrus here caps sync waits per instruction; move excess onto
    InstDrain carriers inserted immediately before, same engine queue."""
    k = 0
    for blk in nc.m.functions[0].blocks:
        il = blk.instructions
        i = 0
        while i < len(il):
            inst = il[i]
            cap = 1 if isinstance(inst, mybir.InstMatmult) else max_waits
            si = getattr(inst, "sync_info", None)
            waits = list(si.on_wait) if si is not None and si.on_wait else []
            if len(waits) > cap:
                si.on_wait = waits[-cap:]
                extras = waits[:-cap]
                pos = i
                for j in range(0, len(extras), max_waits):
                    d = mybir.InstDrain(name=f"waitnop_{k}", ins=[], outs=[])
                    k += 1
                    d.engine = inst.engine
                    d.sync_info = mybir.SyncInfo(
                        on_wait=extras[j : j + max_waits], on_update=[]
                    )
                    il.insert(pos, d)
                    pos += 1
                    i += 1
            i += 1
    return k


def build_nc():
    nc = bass.Bass()
    x_h = nc.declare_dram_parameter("x", [S, D], F32, isOutput=False)
    wq_h = nc.declare_dram_parameter("wq", [D, D], F32R, isOutput=False)
    wk_h = nc.declare_dram_parameter("wk", [D, D], F32R, isOutput=False)
    wv_h = nc.declare_dram_parameter("wv", [D, D], F32R, isOutput=False)
    bq_h = nc.declare_dram_parameter("bq", [D], F32, isOutput=False)
    bk_h = nc.declare_dram_parameter("bk", [D], F32, isOutput=False)
    bv_h = nc.declare_dram_parameter("bv", [D], F32R, isOutput=False)
    out_h = nc.declare_dram_parameter("out", [NQ, D], F32, isOutput=True)
    tau_dram = nc.dram_tensor("tau_scratch", [Q_TILES, P], F32R)

    Ident = mybir.ActivationFunctionType.Identity
    Exp = mybir.ActivationFunctionType.Exp
    ge = mybir.AluOpType.is_ge
    mult = mybir.AluOpType.mult

    with SplitDrainTC(nc) as tc:
        with (
            tc.tile_pool(name="big", bufs=1) as big,
            tc.tile_pool(name="consts", bufs=1) as consts,
            tc.tile_pool(name="wpool", bufs=1) as wpool,
        ):
            # ---- constants ----
            ident = consts.tile([P, P], F32)
            make_identity(nc, ident)
            ones_f32 = consts.tile([1, P], F32)
            nc.vector.memset(ones_f32, 1.0)
            ones_row = consts.tile([1, P], F32R)
            nc.vector.tensor_copy(ones_row[:], ones_f32[:])
            ones_col = consts.tile([P, 2], F32)
            nc.vector.memset(ones_col, 1.0)
            # weights: [128, kt, 256] with row (kt*128+p) -> [p, kt, :]
            w_sb = {}
            for name, h in (("q", wq_h), ("k", wk_h), ("v", wv_h)):
                t = wpool.tile([P, 2, D], F32R, name=f"w{name}", tag=f"w{name}")
                nc.sync.dma_start(
                    out=t[:], in_=h[:].rearrange("(a p) d -> p a d", p=P)
                )
                w_sb[name] = t
            # biases bq/bk: [128, 2] (per-partition cols per d-tile)
            b_sb = {}
            for name, h in (("q", bq_h), ("k", bk_h)):
                t = wpool.tile([P, 2], F32, name=f"b{name}", tag=f"b{name}")
                nc.sync.dma_start(out=t[:], in_=h[:].rearrange("(a p) -> p a", p=P))
                b_sb[name] = t
            # bv as a [1, 256] row (added to V via rank-1 matmul)
            bv_row = consts.tile([1, D], F32R)
            nc.sync.dma_start(out=bv_row[:], in_=bv_h[:].rearrange("(a d) -> a d", a=1))

            # ---- big persistent tensors ----
            XT = [big.tile([P, S], F32R, name=f"XT{i}", tag=f"XT{i}") for i in range(2)]
            KT = [big.tile([P, S], F32R, name=f"KT{i}", tag=f"KT{i}") for i in range(2)]
            QT = [big.tile([P, NQ], F32R, name=f"QT{i}", tag=f"QT{i}") for i in range(2)]
            Vb = big.tile([P, T_TILES, D + 2], F32R, tag="Vb")

            # ---- prologue: load x, transpose to XT ----
            with (
                tc.tile_pool(name="xstage", bufs=4) as xstage,
                tc.tile_pool(name="tpsum", bufs=4, space="PSUM") as tpsum,
            ):
                for tt in range(T_TILES):
                    xt = xstage.tile([P, D], F32)
                    nc.sync.dma_start(out=xt[:], in_=x_h[tt * P : (tt + 1) * P, :])
                    for dh in range(2):
                        tp = tpsum.tile([P, P], F32)
                        nc.tensor.transpose(tp[:], xt[:, dh * P : (dh + 1) * P], ident[:])
                        dst = XT[dh][:, tt * P : (tt + 1) * P]
                        if (tt * 2 + dh) % 2 == 0:
                            nc.scalar.copy(dst, tp[:])
                        else:
                            nc.vector.tensor_copy(dst, tp[:])

            # ---- projections ----
            with tc.tile_pool(name="ppsum", bufs=2, space="PSUM") as ppsum:
                # KT[dt][d, t] and QT[dt][d, q]: lhsT = W rows, rhs = XT
                for (name, dest, ncols) in (("k", KT, S), ("q", QT, NQ)):
                    w = w_sb[name]
                    bcol = b_sb[name]
                    for dt in range(2):
                        for ch in range(ncols // 512):
                            pp = ppsum.tile([P, 512], F32, tag="pp")
                            for kt in range(2):
                                nc.tensor.matmul(
                                    pp[:],
                                    _r(w[:, kt, dt * P : (dt + 1) * P]),
                                    _r(XT[kt][:, ch * 512 : (ch + 1) * 512]),
                                    start=(kt == 0),
                                    stop=(kt == 1),
                                )
                            dst = dest[dt][:, ch * 512 : (ch + 1) * 512]
                            if ch % 2 == 0:
                                nc.scalar.activation(
                                    dst, pp[:], Ident, bias=bcol[:, dt : dt + 1]
                                )
                            else:
                                nc.vector.tensor_scalar_add(
                                    dst, pp[:], bcol[:, dt : dt + 1]
                                )
                # V[t, d] natural layout + ones column; bias via rank-1
                for tt in range(T_TILES):
                    vp = ppsum.tile([P, D], F32, tag="vp")
                    for kt in range(2):
                        nc.tensor.matmul(
                            vp[:],
                            _r(XT[kt][:, tt * P : (tt + 1) * P]),
                            _r(w_sb["v"][:, kt, :]),
                            start=(kt == 0),
                            stop=False,
                        )
                    nc.tensor.matmul(
                        vp[:], _r(ones_row[:]), _r(bv_row[:]), start=False, stop=True
                    )
                    nc.scalar.copy(Vb[:, tt, 0:D], vp[:])
                    nc.vector.tensor_copy(Vb[:, tt, D : D + 2], ones_col[:])

            # ---- main loop ----
            with (
                tc.tile_pool(name="simps", bufs=2, space="PSUM") as simps,
                tc.tile_pool(name="stps", bufs=2, space="PSUM") as stps,
                tc.tile_pool(name="outps", bufs=4, space="PSUM") as outps,
                tc.tile_pool(name="cpool", bufs=3) as cpool,
                tc.tile_pool(name="spool", bufs=10) as spool,
                tc.tile_pool(name="epool", bufs=3) as epool,
                tc.tile_pool(name="ptpool", bufs=3) as ptpool,
                tc.tile_pool(name="osb", bufs=3) as osb,
                tc.tile_pool(name="trow", bufs=2) as trow,
            ):
                for g in range(N_GROUPS):
                    taurow = trow.tile([1, QG * P], F32R)
                    # --- per q-tile: sim + top-32 threshold ---
                    for qi in range(QG):
                        qt = g * QG + qi
                        C = cpool.tile([P, P], F32, tag="C")
                        for ch in range(S // 512):
                            sp = simps.tile([P, 512], F32, tag="sp")
                            for kt in range(2):
                                nc.tensor.matmul(
                                    sp[:],
                                    _r(QT[kt][:, qt * P : (qt + 1) * P]),
                                    _r(KT[kt][:, ch * 512 : (ch + 1) * 512]),
                                    start=(kt == 0),
                                    stop=(kt == 1),
                                )
                            for hh in range(2):
                                j = ch * 2 + hh
                                nc.vector.max(
                                    out=C[:, j * 8 : (j + 1) * 8],
                                    in_=sp[:, hh * 256 : (hh + 1) * 256],
                                )
                        # 4 rounds of top-8 extraction on C
                        cur = C
                        v8 = None
                        for r in range(4):
                            v8 = spool.tile([P, 8], F32, tag="v8")
                            nc.vector.max(out=v8[:], in_=cur[:])
                            if r < 3:
                                nxt = cpool.tile([P, P], F32, tag="C")
                                nc.vector.match_replace(
                                    out=nxt[:],
                                    in_to_replace=v8[:],
                                    in_values=cur[:],
                                    imm_value=NEG_BIG,
                                )
                                cur = nxt
                        tau_neg = spool.tile([P, 1], F32R, tag="tn")
                        nc.vector.tensor_scalar_mul(tau_neg[:], v8[:, 7:8], -1.0)
                        nc.sync.dma_start(
                            out=tau_dram[qt, :].rearrange("(p one) -> p one", one=1),
                            in_=tau_neg[:],
                        )
                        nc.sync.dma_start(
                            out=taurow[0:1, qi * P : (qi + 1) * P],
                            in_=tau_dram[qt, :].rearrange("(a p) -> a p", a=1),
                        )

                    # --- simT + masked exp + PV over t tiles ---
                    outp = [
                        outps.tile([P, D + 2], F32, name="op", tag="op") for _ in range(QG)
                    ]
                    for tt in range(T_TILES):
                        st = stps.tile([P, QG * P], F32, tag="st")
                        for kt in range(2):
                            nc.tensor.matmul(
                                st[:],
                                _r(KT[kt][:, tt * P : (tt + 1) * P]),
                                _r(QT[kt][:, g * QG * P : (g + 1) * QG * P]),
                                start=(kt == 0),
                                stop=False,
                            )
                        nc.tensor.matmul(
                            st[:], _r(ones_row[:]), _r(taurow[:]), start=False,
                            stop=True,
                        )
                        e_t = epool.tile([P, QG * P], F32, tag="e")
                        nc.scalar.activation(e_t[:], st[:], Exp)
                        p_t = ptpool.tile([P, QG * P], F32R, tag="pt")
                        nc.vector.scalar_tensor_tensor(
                            out=p_t[:], in0=e_t[:], scalar=MASK_THRESH,
                            in1=e_t[:], op0=ge, op1=mult,
                        )
                        for qi in range(QG):
                            nc.tensor.matmul(
                                outp[qi][:],
                                _r(p_t[:, qi * P : (qi + 1) * P]),
                                _r(Vb[:, tt, :]),
                                start=(tt == 0),
                                stop=(tt == T_TILES - 1),
                            )
                    # --- normalize + store ---
                    for qi in range(QG):
                        rc = spool.tile([P, 1], F32, tag="rc")
                        nc.vector.reciprocal(rc[:], outp[qi][:, D : D + 1])
                        ob = osb.tile([P, D], F32, tag="ob")
                        nc.vector.tensor_scalar_mul(ob[:], outp[qi][:, 0:D], rc[:])
                        r0 = (g * QG + qi) * P
                        nc.sync.dma_start(out=out_h[r0 : r0 + P, :], in_=ob[:])
    n = _split_excess_waits(nc)
    return nc


_NC_CACHE = None


def kernel(x, Wq, bq, Wk, bk, Wv, bv):
    global _NC_CACHE
    x = np.asarray(x, dtype=np.float32)
    Wq = np.asarray(Wq, dtype=np.float32)
    Wk = np.asarray(Wk, dtype=np.float32)
    Wv = np.asarray(Wv, dtype=np.float32)
    bq = np.asarray(bq, dtype=np.float32)
    bk = np.asarray(bk, dtype=np.float32)
    bv = np.asarray(bv, dtype=np.float32)
    B, S_, D_ = x.shape
    assert (B, S_, D_) == (4, S, D)

    if _NC_CACHE is None:
        _NC_CACHE = build_nc()
    nc = _NC_CACHE

    in_maps = []
    for c in range(8):
        b, h = c // 2, c % 2
        xb = np.roll(x[b], -h * NQ, axis=0)  # queries first, keys permuted
        in_maps.append(
            {"x": np.ascontiguousarray(xb), "wq": Wq, "wk": Wk, "wv": Wv,
             "bq": bq, "bk": bk, "bv": bv}
        )
    res = run_bass_kernel_spmd(nc, in_maps, list(range(8)))
    out = np.empty((B, S, D), dtype=np.float32)
    for c in range(8):
        b, h = c // 2, c % 2
        out[b, h * NQ : (h + 1) * NQ, :] = res.results[c]["out"]
    return out
